# revision 15
# baseline (speedup 1.0000x reference)
"""Causal multi-head attention block (QKV proj + causal softmax attention + out proj)
for Trainium2, sharded over 8 NeuronCores.

Sharding: tensor-parallel over heads x data-parallel over batch.
  core (b, g) for b in {0,1}, g in {0..3}: batch b, head group g (4 heads of 16).
  Each core computes its 4 heads' attention output slice and a partial
  output projection (row-parallel W_O); host sums the 4 partials per batch.

Device layout: everything is computed in "transposed" orientation so no
on-chip transposes are needed anywhere:
  - host passes x^T, W_qkv^T (sliced), W_O^T (sliced) per core
  - Q^T,K^T = (W^T)^T @ x^T via PE;  V in natural [t,d] layout (+ ones
    column written by a gpsimd memset: it makes the softmax denominator
    fall out of the PV matmul for free)
  - S^T[k,q] = (K^T)^T @ Q^T per 128-wide k-tile; two k-tiles of a q-chunk
    share one 2-bank PSUM tile so a single ScalarE exp covers both (halves
    the per-instruction activation overhead); causal masking via gpsimd
    affine_select zeroing after exp + per-k-tile valid-column windows
  - O^T[d,q] (+ sums row from the ones column) = V_aug^T @ P^T accumulated
    over the causally-valid column ranges
  - normalize: stage sums to sbuf p0, DVE reciprocal_approx_fast, gpsimd
    partition_broadcast, one VectorE multiply
  - partial_out[t,o] = (attn^T)^T @ W_O^T, staged [128,1024] in SBUF and
    DMA'd out one t-tile at a time

The emission is software-pipelined at k-tile-pair granularity: the PE
stream is ST(i); fillers; PV(i-1), so the in-order PE queue never parks on
a PV whose exp hasn't finished. Fillers are the next chunk's QKV projection
k-steps and earlier chunks' W_O tiles. Input DMAs go through one queue in
an explicit order: quarter-interleaved W/x-chunk0 (fast PE start), then one
large DMA per remaining x chunk (few HWDGE holds).

All matmul operands use dtype float32r (fp32 bits, PE 'replicated'
datapath): full bf16-rate matmul at ~1e-4 relative accuracy.
"""

import sys

sys.path.insert(0, "/opt/trn_rl_repo")

import numpy as np

import concourse.bacc as bacc
import concourse.mybir as mybir
import concourse.tile as tile
from concourse import bass_utils

B, T, C = 2, 2048, 1024
H, DK = 16, 64
G = 4  # tensor-parallel head groups
HG = H // G  # heads per core
WQK = 2 * HG * DK  # 512: Q+K cols per core in wqkvT
WV = HG * DK  # 256: V cols per core
N_CORES = 8
F32 = mybir.dt.float32
F32R = mybir.dt.float32r

TCH = 4  # t chunks of 512 for N-dim of matmuls
CK = C // 128  # 8 contraction chunks
NT = T // 128  # 16 t-tiles
QCH = 512  # q chunk


def _emit(nc, xT, wqkvT, woT, out):
    with tile.TileContext(nc) as tc:
        with (
            tc.tile_pool(name="persist", bufs=1) as persist,
            tc.tile_pool(name="pt", bufs=3) as pt_pool,
            tc.tile_pool(name="rb", bufs=2) as rb_pool,
            tc.tile_pool(name="ob", bufs=3) as ob_pool,
            tc.tile_pool(name="qkv_ps", bufs=2, space="PSUM") as qkv_ps,
            tc.tile_pool(name="st_ps", bufs=2, space="PSUM") as st_ps,
            tc.tile_pool(name="ot_ps", bufs=2, space="PSUM") as ot_ps,
        ):
            xT_all = persist.tile([128, CK, T], F32R, tag="xT_all")
            w_all = persist.tile([128, CK, 3 * WV], F32R, tag="w_all")
            woT_all = persist.tile([128, 2, C], F32R, tag="woT_all")
            qkT = persist.tile([128, 4, T], F32R, tag="qkT")
            vaug = persist.tile([128, NT, HG, DK + 1], F32R, tag="vaug")
            attnT = persist.tile([128, 2, T], F32R, tag="attnT")

            # ---- input DMA stream (one queue => explicit global order) ----
            for q in range(4):
                ks = slice(q * 256, (q + 1) * 256)
                nc.sync.dma_start(
                    w_all[:, 2 * q : 2 * q + 2, :],
                    wqkvT[ks, :].rearrange("(k p) f -> p k f", p=128),
                )
                nc.sync.dma_start(
                    xT_all[:, 2 * q : 2 * q + 2, 0:QCH],
                    xT[ks, 0:QCH].rearrange("(k p) t -> p k t", p=128),
                )
            # ones column for the softmax-denominator rows.
            # f32r memsets fail the ISA check; same bits via an f32 view.
            nc.gpsimd.memset(vaug[:, :, :, DK : DK + 1].bitcast(F32), 1.0)
            for tch in range(1, TCH):
                nc.sync.dma_start(
                    xT_all[:, :, tch * QCH : (tch + 1) * QCH],
                    xT[:, tch * QCH : (tch + 1) * QCH].rearrange(
                        "(k p) t -> p k t", p=128
                    ),
                )
            nc.sync.dma_start(
                woT_all[:, :, :], woT[:, :].rearrange("(j p) c -> p j c", p=128)
            )

            # ---- QKV projection pieces ----
            def qk_mm(ps, j, tch, k):
                nc.tensor.matmul(
                    ps[:],
                    w_all[:, k, j * 128 : (j + 1) * 128],
                    xT_all[:, k, tch * QCH : (tch + 1) * QCH],
                    start=(k == 0),
                    stop=(k == CK - 1),
                )

            def qk_copy(ps, j, tch):
                nc.vector.tensor_copy(qkT[:, j, tch * QCH : (tch + 1) * QCH], ps[:])

            def v_mm(ps, ti, k):
                nc.tensor.matmul(
                    ps[:],
                    xT_all[:, k, ti * 128 : (ti + 1) * 128],
                    w_all[:, k, WQK : WQK + WV],
                    start=(k == 0),
                    stop=(k == CK - 1),
                )

            def v_copy(ps, ti):
                nc.vector.tensor_copy(
                    vaug[:, ti, :, 0:DK],
                    ps[:].rearrange("p (h d) -> p h d", h=HG),
                )

            def emit_qkv_chunk0():
                # k-outer over the four W tiles plus V(t0),V(t1) -- six live
                # accumulators (qkv, st and ot pools) -- so the PE consumes
                # each arriving quarter of x/w at the DMA's pace. V(t2),V(t3)
                # follow as the first seg-0 fillers.
                ps_j = {}
                for j in (0, 2):
                    ps_j[j] = qkv_ps.tile([128, QCH], F32, tag="mm", name=f"psj{j}")
                for j in (1, 3):
                    big = st_ps.tile([128, 2 * QCH], F32, tag="st", name=f"psj{j}")
                    ps_j[j] = big[:, 0:QCH]
                ps_v = {}
                for ti in (0, 1):
                    ps_v[ti] = ot_ps.tile([128, WV], F32, tag="ot", name=f"psv{ti}")
                for k in range(CK):
                    for j in (0, 2, 1, 3):
                        qk_mm(ps_j[j], j, 0, k)
                    for ti in (0, 1):
                        v_mm(ps_v[ti], ti, k)
                for j in (0, 2, 1, 3):
                    qk_copy(ps_j[j], j, 0)
                for ti in (0, 1):
                    v_copy(ps_v[ti], ti)

            # ---- filler items (PE work with no ACT dependencies) ----
            # each item is a closure emitting ~1-2 matmuls; psum slots are
            # allocated inside the first item so pool rotation follows the
            # actual consumption order.
            def qk_pair_items(ja, jb, tch):
                hold = {}

                def step(k):
                    if k == 0:
                        hold["a"] = qkv_ps.tile([128, QCH], F32, tag="mm", name="qa")
                        hold["b"] = qkv_ps.tile([128, QCH], F32, tag="mm", name="qb")
                    qk_mm(hold["a"], ja, tch, k)
                    qk_mm(hold["b"], jb, tch, k)

                items = [lambda k=k: step(k) for k in range(CK)]
                items.append(
                    lambda: (qk_copy(hold["a"], ja, tch), qk_copy(hold["b"], jb, tch))
                )
                return items

            def v_pair_items(ta, tb):
                hold = {}

                def step(k):
                    if k == 0:
                        hold["a"] = qkv_ps.tile([128, WV], F32, tag="mm", name="va")
                        hold["b"] = qkv_ps.tile([128, WV], F32, tag="mm", name="vb")
                    v_mm(hold["a"], ta, k)
                    v_mm(hold["b"], tb, k)

                items = [lambda k=k: step(k) for k in range(CK)]
                items.append(lambda: (v_copy(hold["a"], ta), v_copy(hold["b"], tb)))
                return items

            def wo_items(ti, tail=False):
                hold = {}
                # in the tail the st pool is idle: rotate through it for 4
                # concurrent W_O psum tiles instead of 2
                use_st = tail and ti % 2 == 1

                def mms(oc):
                    if oc == 0:
                        hold["ob"] = ob_pool.tile([128, C], F32, tag="ob", name="ob")
                    if use_st:
                        big = st_ps.tile([128, 2 * QCH], F32, tag="st", name="wost")
                        hold[oc] = big[:, 0:QCH]
                    else:
                        hold[oc] = qkv_ps.tile([128, QCH], F32, tag="mm", name="wo")
                    for j in range(2):
                        nc.tensor.matmul(
                            hold[oc][:],
                            attnT[:, j, ti * 128 : (ti + 1) * 128],
                            woT_all[:, j, oc * QCH : (oc + 1) * QCH],
                            start=(j == 0),
                            stop=(j == 1),
                        )

                def fin(oc):
                    ob = hold["ob"]
                    use_act = tail and (oc == 0) != (ti % 2 == 1)
                    copy = nc.scalar.copy if use_act else nc.vector.tensor_copy
                    copy(ob[:, oc * QCH : (oc + 1) * QCH], hold[oc][:])
                    if tail:
                        # split DMA halves to shorten the drain
                        nc.sync.dma_start(
                            out[ti * 128 : (ti + 1) * 128, oc * QCH : (oc + 1) * QCH],
                            ob[:, oc * QCH : (oc + 1) * QCH],
                        )
                    elif oc == 1:
                        nc.sync.dma_start(out[ti * 128 : (ti + 1) * 128, :], ob[:])

                return [lambda: mms(0), lambda: mms(1), lambda: fin(0), lambda: fin(1)]

            # ---- attention units ----
            # unit = one 2-bank ST/exp tile holding one or two k-tiles of a
            # (head, q-chunk): (ragged, full) pairs keep the exp window
            # contiguous; leftover raggeds (chunk 0 only) go solo in bank 0.
            def make_units(seg, h):
                # raggeds = the 4 diagonal-band tiles (delta >= 0, need the
                # affine mask, must sit in bank 0); fulls = strictly-below-
                # diagonal tiles (no mask), usable in bank 1.
                fulls = list(range(4 * seg))
                raggeds = [4 * seg, 4 * seg + 1, 4 * seg + 2, 4 * seg + 3]
                units = []
                nrf = min(len(raggeds), len(fulls))
                for i in range(nrf):
                    units.append((seg, h, raggeds[i], fulls[i]))
                rem_f = fulls[nrf:]
                for i in range(0, len(rem_f) - 1, 2):
                    units.append((seg, h, rem_f[i], rem_f[i + 1]))
                for r in raggeds[nrf:]:
                    units.append((seg, h, r, None))
                return units

            def w0_of(seg, k):
                d0 = max(k * 128 - seg * QCH, 0)
                return min(d0, QCH - 256) if d0 else 0

            def emit_st(u):
                seg, h, ka, kb = u
                q0 = seg * QCH
                prow = (h % 2) * 64
                QT_h = qkT[prow : prow + 64, h // 2, :]
                KT_h = qkT[prow : prow + 64, 2 + h // 2, :]
                st = st_ps.tile([128, 2 * QCH], F32, tag="st", name="st")
                w0a = w0_of(seg, ka)
                nc.tensor.matmul(
                    st[:, w0a:QCH],
                    KT_h[:, ka * 128 : (ka + 1) * 128],
                    QT_h[:, q0 + w0a : q0 + QCH],
                    start=True,
                    stop=True,
                )
                if kb is not None:
                    nc.tensor.matmul(
                        st[:, QCH : 2 * QCH],
                        KT_h[:, kb * 128 : (kb + 1) * 128],
                        QT_h[:, q0 : q0 + QCH],
                        start=True,
                        stop=True,
                    )
                return st, w0a

            def emit_exp_mask(u, st, w0a):
                seg, h, ka, kb = u
                q0 = seg * QCH
                hi = 2 * QCH if kb is not None else QCH
                pt = pt_pool.tile([128, 2 * QCH], F32R, tag="pt", name="pt")
                nc.scalar.activation(
                    pt[:, w0a:hi],
                    st[:, w0a:hi],
                    mybir.ActivationFunctionType.Exp,
                    scale=float(1.0 / np.sqrt(DK)),
                )
                delta = ka * 128 - q0
                if delta >= 0:
                    aw = delta + 128 - w0a
                    nc.gpsimd.affine_select(
                        out=pt[:, w0a : w0a + aw],
                        in_=pt[:, w0a : w0a + aw],
                        compare_op=mybir.AluOpType.is_ge,
                        fill=0.0,
                        base=w0a - delta,
                        pattern=[[1, aw]],
                        channel_multiplier=-1,
                    )
                return pt

            def emit_pv(u, pt, w0a, ot, first, last):
                seg, h, ka, kb = u
                if kb is not None:
                    # full tile in bank 1 first: the head-chunk's first PV
                    # carries start=True over the whole [0:QCH] range
                    nc.tensor.matmul(
                        ot[:, 0:QCH],
                        vaug[:, kb, h, :],
                        pt[:, QCH : 2 * QCH],
                        start=first,
                        stop=False,
                    )
                    first = False
                nc.tensor.matmul(
                    ot[:, w0a:QCH],
                    vaug[:, ka, h, :],
                    pt[:, w0a:QCH],
                    start=first,
                    stop=last,
                )

            def emit_norm(u, ot):
                seg, h, _, _ = u
                q0 = seg * QCH
                prow = (h % 2) * 64
                # the custom-DVE recip needs a base-partition-0 source:
                # stage the psum sums row (partition 64) through sbuf p0
                sums = rb_pool.tile([1, QCH], F32, tag="sums", name="sums")
                nc.vector.tensor_copy(sums[:], ot[DK : DK + 1, :])
                recip = rb_pool.tile([1, QCH], F32, tag="recip", name="recip")
                nc.vector.reciprocal_approx_fast(out=recip[:], in_=sums[:])
                rb = rb_pool.tile([64, QCH], F32, tag="rb", name="rb")
                nc.gpsimd.partition_broadcast(rb[:], recip[:])
                nc.vector.tensor_tensor(
                    attnT[prow : prow + 64, h // 2, q0 : q0 + QCH],
                    ot[0:DK, :],
                    rb[:],
                    mybir.AluOpType.mult,
                )

            # ---- the pipelined schedule ----
            emit_qkv_chunk0()

            wo_map = {1: [0, 1, 2, 3], 2: [4, 5], 3: [6, 7, 8, 9, 10, 11]}
            pending = None  # (u, pt, w0a, first, last)
            pend_fillers = []
            ot_hold = {}

            def flush():
                nonlocal pending
                if pending is None:
                    return
                u, pt, w0a, first, last = pending
                for f in pend_fillers:
                    f()
                pend_fillers.clear()
                key = (u[0], u[1])
                if first:
                    ot_hold[key] = ot_ps.tile(
                        [DK + 1, QCH], F32, tag="ot", name="ot"
                    )
                ot = ot_hold[key]
                emit_pv(u, pt, w0a, ot, first, last)
                if last:
                    emit_norm(u, ot)
                    del ot_hold[key]
                pending = None

            for seg in range(TCH):
                units = []
                for h in range(HG):
                    us = make_units(seg, h)
                    for i, u in enumerate(us):
                        units.append((u, i == 0, i == len(us) - 1))
                fillers = []
                early = v_pair_items(2, 3) if seg == 0 else []
                nxt = seg + 1
                if nxt < TCH:
                    fillers += qk_pair_items(0, 1, nxt)
                    fillers += qk_pair_items(2, 3, nxt)
                    fillers += v_pair_items(4 * nxt, 4 * nxt + 1)
                    fillers += v_pair_items(4 * nxt + 2, 4 * nxt + 3)
                for ti in wo_map.get(seg, ()):
                    fillers += wo_items(ti)
                n = len(units)
                # chunk 0: x chunk1 is still in flight for the first units, but
                # V(t2),V(t3) (chunk-0 data) must complete by unit 2 -- their
                # PVs read vaug[2],vaug[3] from unit 2 on.
                start_at = n // 2 if seg == 0 else 0
                m = n - start_at
                for i, (u, first, last) in enumerate(units):
                    st, w0a = emit_st(u)
                    pt = emit_exp_mask(u, st, w0a)
                    flush()
                    if i < 3:
                        pend_fillers.extend(early[3 * i : 3 * i + 3])
                    if i >= start_at:
                        j = i - start_at
                        pend_fillers.extend(
                            fillers[j * len(fillers) // m : (j + 1) * len(fillers) // m]
                        )
                    pending = (u, pt, w0a, first, last)
            flush()
            # tail: the last chunk's W_O tiles, DMA'd in halves
            for ti in range(4 * (TCH - 1), 4 * TCH):
                for f in wo_items(ti, tail=True):
                    f()


_CACHE = {}


def _build():
    if "nc" in _CACHE:
        return _CACHE["nc"]
    nc = bacc.Bacc("TRN2", debug=False, num_devices=N_CORES)
    xT = nc.dram_tensor("xT", [C, T], F32R, kind="ExternalInput").ap()
    wqkvT = nc.dram_tensor("wqkvT", [C, 3 * WV], F32R, kind="ExternalInput").ap()
    woT = nc.dram_tensor("woT", [2 * 128, C], F32R, kind="ExternalInput").ap()
    out = nc.dram_tensor("out", [T, C], F32, kind="ExternalOutput").ap()
    _emit(nc, xT, wqkvT, woT, out)
    nc.compile()
    _CACHE["nc"] = nc
    return nc


def _shard_inputs(x, W_QKV, W_O):
    """Build the 8 per-core input maps. core = b*G + g."""
    in_maps = []
    W_Q, W_K, W_V = W_QKV[0:C], W_QKV[C : 2 * C], W_QKV[2 * C : 3 * C]
    for b in range(B):
        xT_b = np.ascontiguousarray(x[b].T)  # [C, T]
        for g in range(G):
            sl = slice(g * HG * DK, (g + 1) * HG * DK)
            w_g = np.concatenate([W_Q[sl], W_K[sl], W_V[sl]], axis=0)  # [768, C]
            wqkvT_g = np.ascontiguousarray(w_g.T)  # [C, 768]
            woT_g = np.ascontiguousarray(W_O[:, sl].T)  # [256, C]
            in_maps.append({"xT": xT_b, "wqkvT": wqkvT_g, "woT": woT_g})
    return in_maps


def kernel(x, W_QKV, W_O):
    x = np.asarray(x, dtype=np.float32)
    W_QKV = np.asarray(W_QKV, dtype=np.float32)
    W_O = np.asarray(W_O, dtype=np.float32)
    nc = _build()
    in_maps = _shard_inputs(x, W_QKV, W_O)
    res = bass_utils.run_bass_kernel_spmd(
        nc, in_maps, core_ids=list(range(N_CORES))
    )
    out = np.zeros((B, T, C), dtype=np.float32)
    for b in range(B):
        for g in range(G):
            out[b] += res.results[b * G + g]["out"]
    return out
